# revision 5
# baseline (speedup 1.0000x reference)
"""Trainium2 Bass kernel for nn_CognitiveWorkspaceTransformer.

Math (reference semantics):
    X   = S + concat(w_spoke, w_hub_priv, w_hub_shared, tag)   # full 1088 cover
    out = X @ W_read.T          # (B,T,1024)
    k   = latent @ Wk.T         # cache is fully overwritten by latent
    v   = latent @ Wv.T

Sharding: data-parallel over batch B=8, one batch element per NeuronCore.
Host side is layout prep only (feature-major transposes, zero-pad to 9*128
rows, slab-major tiling, bf16 downcast) - no arithmetic.

Schedule (all times per core, bf16, PE floor ~150us):
  - slab0 (512 tokens) is "j-streamed": its out-GEMM accumulates into 4
    PSUM tiles wave by wave (2 feature chunks per wave), so the PE starts
    real work ~3us in, as soon as the first 1MB wave lands - no long
    kv-first ramp, no deferred stores.
  - host relayout gives per-partition-contiguous slab loads (9KB lines,
    one DMA per tensor per slab) and 2KB wave lines.
  - k/v units (128 tokens each) are interleaved one-per-group through the
    steady slabs as byte-cheap PE filler; the last slab has none so the
    tail drains fast. Slabs 5/6 take the spillover units.
  - loads: S on sync queue, wc on scalar queue (balanced); stores all on
    the gpsimd queue so they never head-of-line-block loads.
  - copies: k->ACT, v->DVE, out alternates; adds on DVE in 3-chunk slices.
  - stores are slab-sized quads (one DMA for 512 tokens of out/k/v).
  - 8 PSUM banks as one 4-slot rotating pool shared by out and k/v.
"""

import numpy as np
import ml_dtypes

import concourse.bacc as bacc
import concourse.mybir as mybir
import concourse.tile as tile
from concourse.bass_utils import run_bass_kernel_spmd

B, T, D_STATE, D_MODEL, D_LATENT = 8, 4096, 1088, 1024, 128
N_CORES = 8
P = 128
F32 = mybir.dt.float32
BF16 = mybir.dt.bfloat16

NJ = 9                  # 1088 = 8*128 + 64 -> 9 chunks (last is 64 rows)
RW = [128] * 8 + [64]   # rows per chunk
SLAB = 512
NSLAB = T // SLAB       # 8
GPS = SLAB // P         # groups per slab = 4
WAVES = [(0, 2), (2, 4), (4, 6), (6, 8), (8, 9)]  # slab0 j-stream waves

# kv units (128 tokens each) assigned to steady slabs; slab 7 gets none.
KV_ASSIGN = {
    0: [0, 1, 2, 3],
    1: [4, 5, 6, 7],
    2: [8, 9, 10, 11],
    3: [12, 13, 14, 15],
    4: [16, 17, 18, 19],
    5: [20, 21, 22, 23, 28, 29],
    6: [24, 25, 26, 27, 30, 31],
    7: [],
}

_NC_CACHE = {}


def _runs(units):
    """Split a sorted unit list into contiguous runs."""
    runs = []
    for u in units:
        if runs and runs[-1][-1] == u - 1:
            runs[-1].append(u)
        else:
            runs.append([u])
    return runs


def build_nc(mm_dt=BF16, out_dt=BF16, in_bufs=4, out_bufs=2, warmup_mms=4):
    nc = bacc.Bacc("TRN2", target_bir_lowering=False, debug=False,
                   num_devices=N_CORES)

    # host-relayout inputs: st/wct [NSLAB, P, NJ, SLAB] (zero-padded rows),
    # latT [P, T], wrt [P, NJ, D_MODEL], wkvt [P, 2, D_MODEL]
    st_d = nc.dram_tensor("st", [NSLAB, P, NJ, SLAB], mm_dt,
                          kind="ExternalInput").ap()
    wct_d = nc.dram_tensor("wct", [NSLAB, P, NJ, SLAB], mm_dt,
                           kind="ExternalInput").ap()
    latt_d = nc.dram_tensor("latt", [P, T], mm_dt, kind="ExternalInput").ap()
    wrt_d = nc.dram_tensor("wrt", [P, NJ, D_MODEL], mm_dt,
                           kind="ExternalInput").ap()
    wkvt_d = nc.dram_tensor("wkvt", [D_LATENT, 2, D_MODEL], mm_dt,
                            kind="ExternalInput").ap()
    out_d = nc.dram_tensor("out", [T, D_MODEL], out_dt, kind="ExternalOutput").ap()
    k_d = nc.dram_tensor("k", [T, D_MODEL], out_dt, kind="ExternalOutput").ap()
    v_d = nc.dram_tensor("v", [T, D_MODEL], out_dt, kind="ExternalOutput").ap()

    with tile.TileContext(nc) as tc:
        with (
            tc.tile_pool(name="weights", bufs=1) as wpool,
            tc.tile_pool(name="ins", bufs=in_bufs) as inpool,
            tc.tile_pool(name="wcp", bufs=in_bufs) as wcpool,
            tc.tile_pool(name="outs", bufs=out_bufs) as outpool,
            tc.tile_pool(name="psum", bufs=4, space="PSUM") as ppool,
        ):
            wr_t = wpool.tile([P, NJ, D_MODEL], mm_dt, tag="wr")
            wkv_t = wpool.tile([D_LATENT, 2, D_MODEL], mm_dt, tag="wkv")
            lat_t = wpool.tile([P, T], mm_dt, tag="lat")

            # ---- slab0 wave loads: S+wr_even on sync, wc+wr_odd on scalar
            s0 = inpool.tile([P, NJ, SLAB], mm_dt, tag="x", name="x0")
            c0 = wcpool.tile([P, NJ, SLAB], mm_dt, tag="wc", name="wc0")
            for wi, (j0, j1) in enumerate(WAVES):
                nc.sync.dma_start(s0[:, j0:j1, :], st_d[0, :, j0:j1, :])
                nc.scalar.dma_start(c0[:, j0:j1, :], wct_d[0, :, j0:j1, :])
                if wi % 2 == 0:
                    nc.sync.dma_start(wr_t[:, j0:j1, :], wrt_d[:, j0:j1, :])
                else:
                    nc.scalar.dma_start(wr_t[:, j0:j1, :], wrt_d[:, j0:j1, :])

            def load_slab(s):
                xt = inpool.tile([P, NJ, SLAB], mm_dt, tag="x", name="xt")
                wc = wcpool.tile([P, NJ, SLAB], mm_dt, tag="wc", name="wc")
                nc.sync.dma_start(xt[:], st_d[s])
                nc.scalar.dma_start(wc[:], wct_d[s])
                return xt, wc

            # one-time tensors + early slabs; lat interleaved on sync
            nc.scalar.dma_start(wkv_t[:], wkvt_d[:])
            xts = {0: (s0, c0)}
            xts[1] = load_slab(1)
            nc.sync.dma_start(lat_t[:, 0:1024], latt_d[:, 0:1024])
            xts[2] = load_slab(2)
            nc.sync.dma_start(lat_t[:, 1024:2048], latt_d[:, 1024:2048])
            xts[3] = load_slab(3)
            nc.sync.dma_start(lat_t[:, 2048:4096], latt_d[:, 2048:4096])

            # ---- adds: slab0 per-wave on DVE; later slabs in 3 chunks
            for (j0, j1) in WAVES:
                nc.vector.tensor_add(s0[:, j0:j1, :], s0[:, j0:j1, :],
                                     c0[:, j0:j1, :])

            def emit_add(s, ci):
                xt, wc = xts[s]
                j0, j1 = [(0, 3), (3, 6), (6, 9)][ci]
                nc.vector.tensor_add(xt[:, j0:j1, :], xt[:, j0:j1, :],
                                     wc[:, j0:j1, :])

            for ci in range(3):
                emit_add(1, ci)

            # ---- PE warm-up junk matmuls
            scratch = wpool.tile([P, 512], mm_dt, tag="scratch")
            nc.vector.memset(scratch[:], 0.0)
            keep = wpool.tile([1, 8], F32, tag="keep")
            pj = ppool.tile([P, D_MODEL], F32, tag="ps", name="pjunk")
            for _ in range(warmup_mms):
                nc.tensor.matmul(pj[:, 0:512], scratch[:, 0:P],
                                 scratch[:, 0:512], start=True, stop=True)
            nc.vector.tensor_copy(keep[:], pj[0:1, 0:8])

            # ---- slab0 j-streamed out-GEMM into 4 psum tiles
            pg = [ppool.tile([P, D_MODEL], F32, tag="ps", name=f"pg{g}")
                  for g in range(GPS)]
            for (j0, j1) in WAVES:
                for j in range(j0, j1):
                    for g in range(GPS):
                        for h in range(2):
                            nc.tensor.matmul(
                                pg[g][:, h * 512: h * 512 + 512],
                                s0[0:RW[j], j, g * P: g * P + P],
                                wr_t[0:RW[j], j, h * 512: h * 512 + 512],
                                start=(j == 0), stop=(j == NJ - 1),
                            )

            def emit_out_group(s, g, dst, eng):
                xt = xts[s][0]
                po = ppool.tile([P, D_MODEL], F32, tag="ps", name="po")
                for j in range(NJ):
                    for h in range(2):
                        nc.tensor.matmul(
                            po[:, h * 512: h * 512 + 512],
                            xt[0:RW[j], j, g * P: g * P + P],
                            wr_t[0:RW[j], j, h * 512: h * 512 + 512],
                            start=(j == 0), stop=(j == NJ - 1),
                        )
                if eng is None:
                    pass
                elif eng is nc.scalar:
                    nc.scalar.copy(dst, po[:])
                else:
                    nc.vector.tensor_copy(dst, po[:])
                return po

            def emit_kv(u, kdst, vdst):
                t0 = u * P
                pk = ppool.tile([P, D_MODEL], F32, tag="ps", name="pk")
                for h in range(2):
                    nc.tensor.matmul(pk[:, h * 512: h * 512 + 512],
                                     lat_t[:, t0: t0 + P],
                                     wkv_t[:, 0, h * 512: h * 512 + 512],
                                     start=True, stop=True)
                nc.scalar.copy(kdst, pk[:])
                pv = ppool.tile([P, D_MODEL], F32, tag="ps", name="pv")
                for h in range(2):
                    nc.tensor.matmul(pv[:, h * 512: h * 512 + 512],
                                     lat_t[:, t0: t0 + P],
                                     wkv_t[:, 1, h * 512: h * 512 + 512],
                                     start=True, stop=True)
                nc.vector.tensor_copy(vdst, pv[:])

            def kv_tiles(nu):
                kq = outpool.tile([P, nu, D_MODEL], out_dt, tag="k",
                                  name="kq", padded_shape=[P, 6, D_MODEL])
                vq = outpool.tile([P, nu, D_MODEL], out_dt, tag="v",
                                  name="vq", padded_shape=[P, 6, D_MODEL])
                return kq, vq

            def store_quad(dram, tl, row0, ng, eng=None):
                (eng or nc.gpsimd).dma_start(
                    dram[row0: row0 + ng * P, :].rearrange(
                        "(g p) d -> p g d", p=P),
                    tl[:])

            def store_kv_runs(units, kq, vq):
                for run in _runs(units):
                    i0 = units.index(run[0])
                    nc.gpsimd.dma_start(
                        k_d[run[0] * P: run[0] * P + len(run) * P, :]
                        .rearrange("(g p) d -> p g d", p=P),
                        kq[:, i0: i0 + len(run), :])
                    nc.gpsimd.dma_start(
                        v_d[run[0] * P: run[0] * P + len(run) * P, :]
                        .rearrange("(g p) d -> p g d", p=P),
                        vq[:, i0: i0 + len(run), :])

            # slab0: copies + kv u0-3 + stores
            oq0 = outpool.tile([P, GPS, D_MODEL], out_dt, tag="out", name="oq0")
            for g in range(GPS):
                if g % 2 == 0:
                    nc.scalar.copy(oq0[:, g, :], pg[g][:])
                else:
                    nc.vector.tensor_copy(oq0[:, g, :], pg[g][:])
            units0 = KV_ASSIGN[0]
            kq0, vq0 = kv_tiles(len(units0))
            for i, u in enumerate(units0):
                emit_kv(u, kq0[:, i, :], vq0[:, i, :])
            store_quad(out_d, oq0, 0, GPS)
            store_kv_runs(units0, kq0, vq0)

            # ---- steady slabs 1..7
            for s in range(1, NSLAB):
                units = KV_ASSIGN[s]
                # prefetch + adds for slab s+1 before this slab's stores
                if s + 1 < NSLAB:
                    xts[s + 1] = load_slab(s + 1)
                    for ci in range(3):
                        emit_add(s + 1, ci)

                oq = outpool.tile([P, GPS, D_MODEL], out_dt, tag="out",
                                  name="oq")
                nu = len(units)
                if nu:
                    kq, vq = kv_tiles(nu)
                ui = 0
                last_slab = (s == NSLAB - 1)
                for g in range(GPS):
                    if last_slab and g == GPS - 1:
                        # final group: split copy across both engines
                        po = emit_out_group(s, g, None, None)
                        nc.scalar.copy(oq[:, g, 0:512], po[:, 0:512])
                        nc.vector.tensor_copy(oq[:, g, 512:1024],
                                              po[:, 512:1024])
                    else:
                        emit_out_group(s, g, oq[:, g, :],
                                       nc.scalar if g % 2 == 0 else nc.vector)
                    # attach kv units round-robin (slabs 5/6 carry 6 units)
                    take = (nu - ui + (GPS - 1 - g)) // (GPS - g)
                    for _ in range(take):
                        emit_kv(units[ui], kq[:, ui, :], vq[:, ui, :])
                        ui += 1
                    if last_slab:
                        # per-group stores for a fast drain
                        eng = nc.gpsimd if g < GPS - 1 else nc.sync
                        store_quad(out_d, oq[:, g: g + 1, :], (s * GPS + g) * P,
                                   1, eng)
                t0 = s * SLAB
                if not last_slab:
                    store_quad(out_d, oq, t0, GPS)
                if nu:
                    store_kv_runs(units, kq, vq)

    nc.compile()
    return nc


def _get_nc(**kw):
    key = tuple(sorted(kw.items()))
    if key not in _NC_CACHE:
        _NC_CACHE[key] = build_nc(**kw)
    return _NC_CACHE[key]


def make_in_maps(S, w_spoke, w_hub_priv, w_hub_shared, tag, W_read, cache,
                 latent, Wk, Wv):
    # host-side layout prep only (shard over batch, feature-major transpose,
    # zero-pad to 9*128 rows, slab-major tiling, bf16 downcast)
    bf = ml_dtypes.bfloat16
    NR = NJ * P  # 1152 padded rows

    def slabify(xT_pad):  # [NR, T] -> [NSLAB, P, NJ, SLAB]
        return np.ascontiguousarray(
            xT_pad.reshape(NJ, P, NSLAB, SLAB).transpose(2, 1, 0, 3)
        ).astype(bf)

    wcat = np.concatenate(
        [np.asarray(w_spoke, np.float32), np.asarray(w_hub_priv, np.float32),
         np.asarray(w_hub_shared, np.float32), np.asarray(tag, np.float32)],
        axis=-1)
    S = np.asarray(S, np.float32)

    sT = np.zeros((B, NR, T), np.float32)
    sT[:, :D_STATE, :] = S.transpose(0, 2, 1)
    wT = np.zeros((B, NR, T), np.float32)
    wT[:, :D_STATE, :] = wcat.transpose(0, 2, 1)

    wr_pad = np.zeros((NR, D_MODEL), np.float32)
    wr_pad[:D_STATE] = np.asarray(W_read, np.float32).T
    wrt = np.ascontiguousarray(
        wr_pad.reshape(NJ, P, D_MODEL).transpose(1, 0, 2)).astype(bf)

    latT = np.ascontiguousarray(
        np.asarray(latent, np.float32).transpose(0, 2, 1)).astype(bf)
    wkvt = np.ascontiguousarray(
        np.stack([np.asarray(Wk, np.float32).T, np.asarray(Wv, np.float32).T],
                 axis=1)).astype(bf)
    return [
        {"st": slabify(sT[i]), "wct": slabify(wT[i]), "latt": latT[i],
         "wrt": wrt, "wkvt": wkvt}
        for i in range(N_CORES)
    ]


def kernel(S, w_spoke, w_hub_priv, w_hub_shared, tag, W_read, cache, latent,
           Wk, Wv, **build_kw):
    in_maps = make_in_maps(S, w_spoke, w_hub_priv, w_hub_shared, tag, W_read,
                           cache, latent, Wk, Wv)
    nc = _get_nc(**build_kw)
    res = run_bass_kernel_spmd(nc, in_maps, list(range(N_CORES)))
    out = np.stack([res.results[i]["out"].astype(np.float32)
                    for i in range(N_CORES)])
    k = np.stack([res.results[i]["k"].astype(np.float32)
                  for i in range(N_CORES)])
    v = np.stack([res.results[i]["v"].astype(np.float32)
                  for i in range(N_CORES)])
    return (out, k, v)


# revision 6
# speedup vs baseline: 1.1848x; 1.1848x over previous
"""Trainium2 Bass kernel for nn_CognitiveWorkspaceTransformer.

Math (reference semantics):
    X   = S + concat(w_spoke, w_hub_priv, w_hub_shared, tag)   # full 1088 cover
    out = X @ W_read.T          # (B,T,1024)
    k   = latent @ Wk.T         # cache is fully overwritten by latent
    v   = latent @ Wv.T

Sharding: data-parallel over batch B=8, one batch element per NeuronCore.
All tensors are laid out feature-major on the host (pure layout prep plus a
bf16 downcast, no arithmetic) so the contraction dim lands on SBUF
partitions directly and the PE needs no on-chip transposes.

bf16 everywhere (tolerance is 2e-2; bf16 lands ~5e-3): ~47MB/core HBM
traffic (~131us roofline) vs ~360k PE cycles (~150us @ 2.4GHz) -> the PE
array is the bottleneck; everything else is scheduled to keep it fed:
  - a few warm-up junk matmuls at t=0 so the HAM clock gate reaches
    2.4GHz before real work lands (cold matmuls run at 1.2GHz)
  - the ramp is ordered for earliest PE start: Wk/Wv (0.5MB) and the
    first 768 latent columns load first, then slab-0/1 S+wc, then the
    bulky W_read; slabs 0-1 are small (256/512) and run their k/v
    matmuls before the first out-matmul; their k/v stores are deferred
    to slabs 2-3 so ramp loads keep the full HBM bandwidth
  - adds are emitted one slab ahead so they never compete with a slab
    tail's PSUM->SBUF copies on the DVE (pout ring would stall the PE)
  - out-copies alternate DVE/ACT per group pair for the same reason
  - slab i+1 loads are issued BEFORE slab i stores: a store waiting on
    its tile would otherwise block later load issues (in-order queues)
  - j-outer/h-inner so each 128x128 stationary X^T chunk is loaded once
  - 2-bank PSUM tiles [128,1024]; ONE wide PSUM->SBUF cast-copy per
    out/k/v tile; paired [256,1024] stores
"""

import numpy as np
import ml_dtypes

import concourse.bacc as bacc
import concourse.mybir as mybir
import concourse.tile as tile
from concourse.bass_utils import run_bass_kernel_spmd

B, T, D_STATE, D_MODEL, D_LATENT = 8, 4096, 1088, 1024, 128
N_CORES = 8
P = 128
F32 = mybir.dt.float32
BF16 = mybir.dt.bfloat16

# feature chunks of the contraction dim (1088 = 8*128 + 64)
R_CHUNKS = [(j * 128, min(128, D_STATE - j * 128)) for j in range((D_STATE + 127) // 128)]
NJ = len(R_CHUNKS)

_NC_CACHE = {}

SLABS = [256, 768, 1024, 1024, 1024]
KV_FIRST = 2      # leading slabs fully k/v-first (deferred stores)
KV2_GROUPS = 0    # first groups of slab 2 also k/v-first
LAT_SPLIT = 1024  # latent columns loaded in the first (small) piece


def build_nc(mm_dt=BF16, out_dt=BF16, in_bufs=3, wc_bufs=2, out_bufs=2,
             warmup_mms=10):
    """Build + compile the per-core Bass program (identical on all cores)."""
    assert sum(SLABS) == T
    max_sz = max(SLABS)

    nc = bacc.Bacc("TRN2", target_bir_lowering=False, debug=False, num_devices=N_CORES)

    # feature-major inputs: sT/wcT [1088, T], latT [128, T], wkvt [128,2,1024]
    st_d = nc.dram_tensor("st", [D_STATE, T], mm_dt, kind="ExternalInput").ap()
    wct_d = nc.dram_tensor("wct", [D_STATE, T], mm_dt, kind="ExternalInput").ap()
    latt_d = nc.dram_tensor("latt", [D_LATENT, T], mm_dt, kind="ExternalInput").ap()
    wrt_d = nc.dram_tensor("wrt", [D_STATE, D_MODEL], mm_dt, kind="ExternalInput").ap()
    wkvt_d = nc.dram_tensor("wkvt", [D_LATENT, 2, D_MODEL], mm_dt,
                            kind="ExternalInput").ap()
    out_d = nc.dram_tensor("out", [T, D_MODEL], out_dt, kind="ExternalOutput").ap()
    k_d = nc.dram_tensor("k", [T, D_MODEL], out_dt, kind="ExternalOutput").ap()
    v_d = nc.dram_tensor("v", [T, D_MODEL], out_dt, kind="ExternalOutput").ap()

    with tile.TileContext(nc) as tc:
        with (
            tc.tile_pool(name="weights", bufs=1) as wpool,
            tc.tile_pool(name="ins", bufs=in_bufs) as inpool,
            tc.tile_pool(name="wcp", bufs=wc_bufs) as wcpool,
            tc.tile_pool(name="outs", bufs=out_bufs) as outpool,
            tc.tile_pool(name="kv0", bufs=4) as kv0pool,
            tc.tile_pool(name="psum_out", bufs=2, space="PSUM") as pout_pool,
            tc.tile_pool(name="psum_kv", bufs=2, space="PSUM") as pkv_pool,
        ):
            # scalar queue: wkv + the first latent piece lead -> k/v matmuls
            # start ~9us in, while W_read/S/wc still stream
            wkv_t = wpool.tile([D_LATENT, 2, D_MODEL], mm_dt, tag="wkv")
            nc.scalar.dma_start(wkv_t[:], wkvt_d[:])
            lt = wpool.tile([D_LATENT, T], mm_dt, tag="lt")
            nc.scalar.dma_start(lt[:, 0:LAT_SPLIT], latt_d[:, 0:LAT_SPLIT])
            ltr = lt[:]

            def issue_loads(it):
                sz = SLABS[it]
                t0 = sum(SLABS[:it])
                xt = inpool.tile([P, NJ, sz], mm_dt, tag="x", name="xt",
                                 padded_shape=[P, NJ, max_sz])
                wc = wcpool.tile([P, NJ, sz], mm_dt, tag="wc", name="wc",
                                 padded_shape=[P, NJ, max_sz])
                nc.sync.dma_start(
                    xt[:, 0:8, :],
                    st_d[0:1024, t0 : t0 + sz].rearrange("(j p) t -> p j t", p=P))
                nc.sync.dma_start(xt[0:64, 8, :], st_d[1024:1088, t0 : t0 + sz])
                nc.scalar.dma_start(
                    wc[:, 0:8, :],
                    wct_d[0:1024, t0 : t0 + sz].rearrange("(j p) t -> p j t", p=P))
                nc.scalar.dma_start(wc[0:64, 8, :], wct_d[1024:1088, t0 : t0 + sz])
                return xt, wc

            def emit_adds(xt, wc, sz):
                xr = xt[:]
                for g in range(sz // P):
                    sl = slice(g * P, (g + 1) * P)
                    nc.vector.tensor_add(xr[:, :, sl], xt[:, :, sl], wc[:, :, sl])
                return xr

            # slab 0 inputs lead, then W_read split across BOTH queues so
            # each queue carries ~half of the out-critical ramp bytes
            slab_tiles = {0: issue_loads(0)}
            wr_all = wpool.tile([P, NJ, D_MODEL], mm_dt, tag="wr")
            nc.sync.dma_start(
                wr_all[:, 0:5, :],
                wrt_d[0:640, :].rearrange("(j p) n -> p j n", p=P))
            nc.scalar.dma_start(
                wr_all[:, 5:8, :],
                wrt_d[640:1024, :].rearrange("(j p) n -> p j n", p=P))
            nc.scalar.dma_start(wr_all[0:64, 8, :], wrt_d[1024:1088, :])
            slab_tiles[1] = issue_loads(1)
            nc.scalar.dma_start(lt[:, LAT_SPLIT:T], latt_d[:, LAT_SPLIT:T])

            # adds for slabs 0 and 1 (DVE picks them up as the loads land)
            xr0 = emit_adds(*slab_tiles[0], SLABS[0])
            xr1 = emit_adds(*slab_tiles[1], SLABS[1])
            xrs = {0: xr0, 1: xr1}

            # HAM warm-up + gap fillers: junk matmuls on a zeroed scratch
            # tile keep the PE clock-gate at 2.4GHz through load waits (an
            # idle window >3.4us halves the PE clock for the next ~4-7us)
            scratch = wpool.tile([P, 512], mm_dt, tag="scratch")
            nc.vector.memset(scratch[:], 0.0)
            keep = wpool.tile([1, 8], F32, tag="keep")

            def junk_fill(n):
                if n <= 0:
                    return
                pj = pout_pool.tile([P, D_MODEL], F32, tag="pout", name="pjunk")
                for w in range(n):
                    nc.tensor.matmul(pj[:, 0:512], scratch[:, 0:P],
                                     scratch[:, 0:512], start=True, stop=True)
                # keep the junk matmuls live
                nc.vector.tensor_copy(keep[:], pj[0:1, 0:8])

            junk_fill(warmup_mms)

            def emit_kv(ts_abs, k_sb, v_sb, pool=None):
                pool = pool or pkv_pool
                tg = "pkv" if pool is pkv_pool else "pout"
                pk = pool.tile([P, D_MODEL], F32, tag=tg, name="pk")
                for h in range(2):
                    nc.tensor.matmul(
                        pk[:, h * 512 : h * 512 + 512],
                        ltr[:, ts_abs : ts_abs + P],
                        wkv_t[:, 0, h * 512 : h * 512 + 512],
                        start=True, stop=True)
                nc.scalar.copy(k_sb, pk[:])
                pv = pool.tile([P, D_MODEL], F32, tag=tg, name="pv")
                for h in range(2):
                    nc.tensor.matmul(
                        pv[:, h * 512 : h * 512 + 512],
                        ltr[:, ts_abs : ts_abs + P],
                        wkv_t[:, 1, h * 512 : h * 512 + 512],
                        start=True, stop=True)
                nc.vector.tensor_copy(v_sb, pv[:])

            def emit_out(xr, ts0, out_sb, copy_eng):
                po = pout_pool.tile([P, D_MODEL], F32, tag="pout", name="po")
                for j, (r0, rw) in enumerate(R_CHUNKS):
                    for h in range(2):
                        nc.tensor.matmul(
                            po[:, h * 512 : h * 512 + 512],
                            xr[0:rw, j, ts0 : ts0 + P],
                            wr_all[0:rw, j, h * 512 : h * 512 + 512],
                            start=(j == 0),
                            stop=(j == NJ - 1),
                        )
                if copy_eng is nc.scalar:
                    nc.scalar.copy(out_sb, po[:])
                else:
                    nc.vector.tensor_copy(out_sb, po[:])

            def pair_store(eng, dram, tl, row0):
                dst = dram[row0 : row0 + 2 * P, :].rearrange("(g p) d -> p g d", p=P)
                eng.dma_start(dst, tl[:])

            # ---- ramp: k/v for slabs 0-1 (stores deferred), then their out
            deferred_kv = []
            ramp_kv = [(it, SLABS[it] // P) for it in range(KV_FIRST)]
            ramp_kv.append((KV_FIRST, KV2_GROUPS))
            for it, ngr in ramp_kv:
                t0 = sum(SLABS[:it])
                kp = vp = None
                for g in range(ngr):
                    if g % 2 == 0:
                        kp = kv0pool.tile([P, 2, D_MODEL], out_dt, tag="k0",
                                          name="k0_pr")
                        vp = kv0pool.tile([P, 2, D_MODEL], out_dt, tag="v0",
                                          name="v0_pr")
                    emit_kv(t0 + g * P, kp[:, g % 2, :], vp[:, g % 2, :])
                    if g % 2 == 1:
                        deferred_kv.append((t0 + (g - 1) * P, kp, vp))

            def emit_out_slab(it):
                t0 = sum(SLABS[:it])
                opair = None
                for g in range(SLABS[it] // P):
                    if g % 2 == 0:
                        opair = outpool.tile([P, 2, D_MODEL], out_dt,
                                             tag="out", name="out_pr")
                    emit_out(xrs[it], g * P, opair[:, g % 2, :],
                             nc.vector if g % 2 == 0 else nc.scalar)
                    if g % 2 == 1:
                        pair_store(nc.scalar if (g // 2) % 2 == 0 else nc.sync,
                                   out_d, opair, t0 + (g - 1) * P)

            emit_out_slab(0)

            # ---- steady slabs ----
            for it in range(1, len(SLABS)):
                sz = SLABS[it]
                t0 = sum(SLABS[:it])

                # prefetch + adds for the NEXT slab before this slab's stores
                if it + 1 < len(SLABS):
                    nxt = issue_loads(it + 1)
                    slab_tiles[it + 1] = nxt
                    xrs[it + 1] = emit_adds(*nxt, SLABS[it + 1])

                # flush deferred ramp k/v stores across slabs 2-3
                if it >= KV_FIRST and deferred_kv:
                    nflush = 2 if it < len(SLABS) - 1 else len(deferred_kv)
                    # (6 ramp pairs total: 2 at it=2, 2 at it=3, rest at it=4)
                    for row0, kp, vp in deferred_kv[:nflush]:
                        pair_store(nc.scalar, k_d, kp, row0)
                        pair_store(nc.sync, v_d, vp, row0)
                    deferred_kv = deferred_kv[nflush:]

                if it < KV_FIRST:
                    emit_out_slab(it)
                    continue

                xr = xrs[it]
                tiles = None
                for g in range(sz // P):
                    if g % 2 == 0:
                        need_kv = not (it == KV_FIRST and g + 1 < KV2_GROUPS)
                        tiles = (
                            outpool.tile([P, 2, D_MODEL], out_dt, tag="out",
                                         name="out_pr"),
                            outpool.tile([P, 2, D_MODEL], out_dt, tag="k",
                                         name="k_pr") if need_kv else None,
                            outpool.tile([P, 2, D_MODEL], out_dt, tag="v",
                                         name="v_pr") if need_kv else None,
                        )
                    kv_inline = not (it == KV_FIRST and g < KV2_GROUPS)
                    last_pair = (it == len(SLABS) - 1) and g >= sz // P - 2
                    if last_pair and g == sz // P - 1:
                        # tail: k/v first so the final PE burst is the out
                        # GEMM and its single copy+store close the kernel
                        emit_kv(t0 + g * P, tiles[1][:, g % 2, :],
                                tiles[2][:, g % 2, :])
                        emit_out(xr, g * P, tiles[0][:, g % 2, :], nc.scalar)
                    else:
                        emit_out(xr, g * P, tiles[0][:, g % 2, :],
                                 nc.vector if g % 2 == 0 else nc.scalar)
                        if kv_inline:
                            emit_kv(t0 + g * P, tiles[1][:, g % 2, :],
                                    tiles[2][:, g % 2, :])
                    if last_pair:
                        # tail: store each final group immediately (unpaired)
                        row0 = t0 + g * P
                        eng = [nc.scalar, nc.sync] if g % 2 == 0 else \
                              [nc.sync, nc.scalar]
                        eng[0].dma_start(out_d[row0 : row0 + P, :],
                                         tiles[0][:, g % 2, :])
                        eng[1].dma_start(k_d[row0 : row0 + P, :],
                                         tiles[1][:, g % 2, :])
                        eng[0].dma_start(v_d[row0 : row0 + P, :],
                                         tiles[2][:, g % 2, :])
                    elif g % 2 == 1:
                        row0 = t0 + (g - 1) * P
                        eng = [nc.scalar, nc.sync] if (g // 2) % 2 == 0 else \
                              [nc.sync, nc.scalar]
                        pair_store(eng[0], out_d, tiles[0], row0)
                        if kv_inline:
                            pair_store(eng[1], k_d, tiles[1], row0)
                            pair_store(eng[0], v_d, tiles[2], row0)

    nc.compile()
    return nc


def _get_nc(**kw):
    key = tuple(sorted(kw.items()))
    if key not in _NC_CACHE:
        _NC_CACHE[key] = build_nc(**kw)
    return _NC_CACHE[key]


def make_in_maps(S, w_spoke, w_hub_priv, w_hub_shared, tag, W_read, cache, latent,
                 Wk, Wv):
    # host-side layout prep only (shard over batch, feature-major transposes,
    # bf16 downcast)
    bf = ml_dtypes.bfloat16
    wcat = np.concatenate(
        [np.asarray(w_spoke, np.float32), np.asarray(w_hub_priv, np.float32),
         np.asarray(w_hub_shared, np.float32), np.asarray(tag, np.float32)],
        axis=-1,
    )
    sT = np.ascontiguousarray(np.asarray(S, np.float32).transpose(0, 2, 1)).astype(bf)
    wcT = np.ascontiguousarray(wcat.transpose(0, 2, 1)).astype(bf)
    latT = np.ascontiguousarray(
        np.asarray(latent, np.float32).transpose(0, 2, 1)).astype(bf)
    wrt = np.ascontiguousarray(np.asarray(W_read, np.float32).T).astype(bf)
    wkvt = np.ascontiguousarray(
        np.stack([np.asarray(Wk, np.float32).T, np.asarray(Wv, np.float32).T],
                 axis=1)).astype(bf)
    return [
        {"st": sT[i], "wct": wcT[i], "latt": latT[i], "wrt": wrt, "wkvt": wkvt}
        for i in range(N_CORES)
    ]


def kernel(S, w_spoke, w_hub_priv, w_hub_shared, tag, W_read, cache, latent, Wk, Wv,
           **build_kw):
    in_maps = make_in_maps(S, w_spoke, w_hub_priv, w_hub_shared, tag, W_read, cache,
                           latent, Wk, Wv)
    nc = _get_nc(**build_kw)
    res = run_bass_kernel_spmd(nc, in_maps, list(range(N_CORES)))
    out = np.stack([res.results[i]["out"].astype(np.float32) for i in range(N_CORES)])
    k = np.stack([res.results[i]["k"].astype(np.float32) for i in range(N_CORES)])
    v = np.stack([res.results[i]["v"].astype(np.float32) for i in range(N_CORES)])
    return (out, k, v)



# revision 10
# speedup vs baseline: 1.1985x; 1.0116x over previous
"""Trainium2 Bass kernel for nn_CognitiveWorkspaceTransformer.

Math (reference semantics):
    X   = S + concat(w_spoke, w_hub_priv, w_hub_shared, tag)   # full 1088 cover
    out = X @ W_read.T          # (B,T,1024)
    k   = latent @ Wk.T         # cache is fully overwritten by latent
    v   = latent @ Wv.T

Sharding: data-parallel over batch B=8, one batch element per NeuronCore.
Host side is layout prep only (feature-major transposes, zero-pad to 9*128
rows, slab-major tiling, bf16 downcast) - no arithmetic.

Schedule (all times per core, bf16, PE floor ~150us):
  - slab0 (512 tokens) is "j-streamed": its out-GEMM accumulates into 4
    PSUM tiles wave by wave (2 feature chunks per wave), so the PE starts
    real work ~3us in, as soon as the first 1MB wave lands - no long
    kv-first ramp, no deferred stores.
  - host relayout gives per-partition-contiguous slab loads (9KB lines,
    one DMA per tensor per slab) and 2KB wave lines.
  - k/v units (128 tokens each) are interleaved one-per-group through the
    steady slabs as byte-cheap PE filler; the last slab has none so the
    tail drains fast. Slabs 5/6 take the spillover units.
  - loads: S on sync queue, wc on scalar queue (balanced); stores all on
    the gpsimd queue so they never head-of-line-block loads.
  - copies: k->ACT, v->DVE, out alternates; adds on DVE in 3-chunk slices.
  - stores are slab-sized quads (one DMA for 512 tokens of out/k/v).
  - 8 PSUM banks as one 4-slot rotating pool shared by out and k/v.
"""

import numpy as np
import ml_dtypes

import concourse.bacc as bacc
import concourse.mybir as mybir
import concourse.tile as tile
from concourse.bass_utils import run_bass_kernel_spmd

B, T, D_STATE, D_MODEL, D_LATENT = 8, 4096, 1088, 1024, 128
N_CORES = 8
P = 128
F32 = mybir.dt.float32
BF16 = mybir.dt.bfloat16

NJ = 9                  # 1088 = 8*128 + 64 -> 9 chunks (last is 64 rows)
RW = [128] * 8 + [64]   # rows per chunk
SLAB = 512
NSLAB = T // SLAB       # 8
GPS = SLAB // P         # groups per slab = 4
WAVES = [(0, 2), (2, 4), (4, 6), (6, 8), (8, 9)]  # slab0 j-stream waves

# kv units (128 tokens each) assigned to steady slabs; slab 7 gets none.
KV_ASSIGN = {
    0: [0, 1, 2, 3],
    1: [4, 5, 6, 7],
    2: [8, 9, 10, 11],
    3: [12, 13, 14, 15],
    4: [16, 17, 18, 19],
    5: [20, 21, 22, 23, 28, 29],
    6: [24, 25, 26, 27, 30, 31],
    7: [],
}

_NC_CACHE = {}


def _runs(units):
    """Split a sorted unit list into contiguous runs."""
    runs = []
    for u in units:
        if runs and runs[-1][-1] == u - 1:
            runs[-1].append(u)
        else:
            runs.append([u])
    return runs


def build_nc(mm_dt=BF16, out_dt=BF16, in_bufs=4, out_bufs=2, warmup_mms=4,
             store_eng="gpsimd"):
    nc = bacc.Bacc("TRN2", target_bir_lowering=False, debug=False,
                   num_devices=N_CORES)

    # host-relayout inputs: st/wct [NSLAB, P, NJ, SLAB] (zero-padded rows),
    # latT [P, T], wrt [P, NJ, D_MODEL], wkvt [P, 2, D_MODEL]
    st_d = nc.dram_tensor("st", [NSLAB, P, NJ, SLAB], mm_dt,
                          kind="ExternalInput").ap()
    wct_d = nc.dram_tensor("wct", [NSLAB, P, NJ, SLAB], mm_dt,
                           kind="ExternalInput").ap()
    latt_d = nc.dram_tensor("latt", [P, T], mm_dt, kind="ExternalInput").ap()
    wrt_d = nc.dram_tensor("wrt", [P, NJ, D_MODEL], mm_dt,
                           kind="ExternalInput").ap()
    wkvt_d = nc.dram_tensor("wkvt", [D_LATENT, 2, D_MODEL], mm_dt,
                            kind="ExternalInput").ap()
    out_d = nc.dram_tensor("out", [T, D_MODEL], out_dt, kind="ExternalOutput").ap()
    k_d = nc.dram_tensor("k", [T, D_MODEL], out_dt, kind="ExternalOutput").ap()
    v_d = nc.dram_tensor("v", [T, D_MODEL], out_dt, kind="ExternalOutput").ap()

    with tile.TileContext(nc) as tc:
        with (
            tc.tile_pool(name="weights", bufs=1) as wpool,
            tc.tile_pool(name="ins", bufs=in_bufs) as inpool,
            tc.tile_pool(name="wcp", bufs=in_bufs) as wcpool,
            tc.tile_pool(name="outs", bufs=out_bufs) as outpool,
            tc.tile_pool(name="psum", bufs=4, space="PSUM") as ppool,
        ):
            wr_t = wpool.tile([P, NJ, D_MODEL], mm_dt, tag="wr")
            wkv_t = wpool.tile([D_LATENT, 2, D_MODEL], mm_dt, tag="wkv")
            lat_t = wpool.tile([P, T], mm_dt, tag="lat")

            # ---- slab0 wave loads: S+wr_even on sync, wc+wr_odd on scalar
            s0 = inpool.tile([P, NJ, SLAB], mm_dt, tag="x", name="x0")
            c0 = wcpool.tile([P, NJ, SLAB], mm_dt, tag="wc", name="wc0")
            for wi, (j0, j1) in enumerate(WAVES):
                nc.sync.dma_start(s0[:, j0:j1, :], st_d[0, :, j0:j1, :])
                nc.scalar.dma_start(c0[:, j0:j1, :], wct_d[0, :, j0:j1, :])
                if wi % 2 == 0:
                    nc.sync.dma_start(wr_t[:, j0:j1, :], wrt_d[:, j0:j1, :])
                else:
                    nc.scalar.dma_start(wr_t[:, j0:j1, :], wrt_d[:, j0:j1, :])

            def load_slab(s):
                xt = inpool.tile([P, NJ, SLAB], mm_dt, tag="x", name="xt")
                wc = wcpool.tile([P, NJ, SLAB], mm_dt, tag="wc", name="wc")
                nc.sync.dma_start(xt[:], st_d[s])
                nc.scalar.dma_start(wc[:], wct_d[s])
                return xt, wc

            # one-time tensors + early slabs; lat interleaved on sync
            nc.scalar.dma_start(wkv_t[:], wkvt_d[:])
            xts = {0: (s0, c0)}
            xts[1] = load_slab(1)
            nc.sync.dma_start(lat_t[:, 0:1024], latt_d[:, 0:1024])
            xts[2] = load_slab(2)
            nc.sync.dma_start(lat_t[:, 1024:2048], latt_d[:, 1024:2048])
            xts[3] = load_slab(3)
            nc.sync.dma_start(lat_t[:, 2048:4096], latt_d[:, 2048:4096])

            # ---- adds: slab0 per-wave on DVE; later slabs in 3 chunks
            for (j0, j1) in WAVES:
                nc.vector.tensor_add(s0[:, j0:j1, :], s0[:, j0:j1, :],
                                     c0[:, j0:j1, :])

            def emit_add(s, ci):
                xt, wc = xts[s]
                j0, j1 = [(0, 3), (3, 6), (6, 9)][ci]
                nc.vector.tensor_add(xt[:, j0:j1, :], xt[:, j0:j1, :],
                                     wc[:, j0:j1, :])

            for ci in range(3):
                emit_add(1, ci)

            # ---- PE warm-up junk matmuls
            scratch = wpool.tile([P, 512], mm_dt, tag="scratch")
            nc.vector.memset(scratch[:], 0.0)
            keep = wpool.tile([1, 8], F32, tag="keep")
            pj = ppool.tile([P, D_MODEL], F32, tag="ps", name="pjunk")
            for _ in range(warmup_mms):
                nc.tensor.matmul(pj[:, 0:512], scratch[:, 0:P],
                                 scratch[:, 0:512], start=True, stop=True)
            nc.vector.tensor_copy(keep[:], pj[0:1, 0:8])

            # ---- slab0 j-streamed out-GEMM into 4 psum tiles
            pg = [ppool.tile([P, D_MODEL], F32, tag="ps", name=f"pg{g}")
                  for g in range(GPS)]
            for (j0, j1) in WAVES:
                for j in range(j0, j1):
                    for g in range(GPS):
                        for h in range(2):
                            nc.tensor.matmul(
                                pg[g][:, h * 512: h * 512 + 512],
                                s0[0:RW[j], j, g * P: g * P + P],
                                wr_t[0:RW[j], j, h * 512: h * 512 + 512],
                                start=(j == 0), stop=(j == NJ - 1),
                            )

            def emit_out_group(s, g, dst, eng):
                xt = xts[s][0]
                po = ppool.tile([P, D_MODEL], F32, tag="ps", name="po")
                for j in range(NJ):
                    for h in range(2):
                        nc.tensor.matmul(
                            po[:, h * 512: h * 512 + 512],
                            xt[0:RW[j], j, g * P: g * P + P],
                            wr_t[0:RW[j], j, h * 512: h * 512 + 512],
                            start=(j == 0), stop=(j == NJ - 1),
                        )
                if eng is None:
                    pass
                elif eng is nc.scalar:
                    nc.scalar.copy(dst, po[:])
                else:
                    nc.vector.tensor_copy(dst, po[:])
                return po

            def emit_kv(u, kdst, vdst):
                t0 = u * P
                pk = ppool.tile([P, D_MODEL], F32, tag="ps", name="pk")
                for h in range(2):
                    nc.tensor.matmul(pk[:, h * 512: h * 512 + 512],
                                     lat_t[:, t0: t0 + P],
                                     wkv_t[:, 0, h * 512: h * 512 + 512],
                                     start=True, stop=True)
                nc.scalar.copy(kdst, pk[:])
                pv = ppool.tile([P, D_MODEL], F32, tag="ps", name="pv")
                for h in range(2):
                    nc.tensor.matmul(pv[:, h * 512: h * 512 + 512],
                                     lat_t[:, t0: t0 + P],
                                     wkv_t[:, 1, h * 512: h * 512 + 512],
                                     start=True, stop=True)
                nc.vector.tensor_copy(vdst, pv[:])

            def kv_tiles(nu):
                kq = outpool.tile([P, nu, D_MODEL], out_dt, tag="k",
                                  name="kq", padded_shape=[P, 6, D_MODEL])
                vq = outpool.tile([P, nu, D_MODEL], out_dt, tag="v",
                                  name="vq", padded_shape=[P, 6, D_MODEL])
                return kq, vq

            _store_rr = [0]
            _store_engs = ({"gpsimd": [nc.gpsimd]}).get(
                store_eng, [nc.sync, nc.scalar])

            def _seng():
                e = _store_engs[_store_rr[0] % len(_store_engs)]
                _store_rr[0] += 1
                return e

            def store_quad(dram, tl, row0, ng, eng=None):
                (eng or _seng()).dma_start(
                    dram[row0: row0 + ng * P, :].rearrange(
                        "(g p) d -> p g d", p=P),
                    tl[:])

            def store_kv_runs(units, kq, vq):
                for run in _runs(units):
                    i0 = units.index(run[0])
                    _seng().dma_start(
                        k_d[run[0] * P: run[0] * P + len(run) * P, :]
                        .rearrange("(g p) d -> p g d", p=P),
                        kq[:, i0: i0 + len(run), :])
                    _seng().dma_start(
                        v_d[run[0] * P: run[0] * P + len(run) * P, :]
                        .rearrange("(g p) d -> p g d", p=P),
                        vq[:, i0: i0 + len(run), :])

            # slab0: copies + kv u0-3 + stores
            oq0 = outpool.tile([P, GPS, D_MODEL], out_dt, tag="out", name="oq0")
            for g in range(GPS):
                if g % 2 == 0:
                    nc.scalar.copy(oq0[:, g, :], pg[g][:])
                else:
                    nc.vector.tensor_copy(oq0[:, g, :], pg[g][:])
            units0 = KV_ASSIGN[0]
            kq0, vq0 = kv_tiles(len(units0))
            for i, u in enumerate(units0):
                emit_kv(u, kq0[:, i, :], vq0[:, i, :])
            store_quad(out_d, oq0, 0, GPS)
            store_kv_runs(units0, kq0, vq0)

            # ---- steady slabs 1..7
            for s in range(1, NSLAB):
                units = KV_ASSIGN[s]
                # prefetch + adds for slab s+1 before this slab's stores
                if s + 1 < NSLAB:
                    xts[s + 1] = load_slab(s + 1)
                    for ci in range(3):
                        emit_add(s + 1, ci)

                oq = outpool.tile([P, GPS, D_MODEL], out_dt, tag="out",
                                  name="oq")
                nu = len(units)
                if nu:
                    kq, vq = kv_tiles(nu)
                ui = 0
                last_slab = (s == NSLAB - 1)
                for g in range(GPS):
                    if last_slab and g == GPS - 1:
                        # final group: split copy across both engines
                        po = emit_out_group(s, g, None, None)
                        nc.scalar.copy(oq[:, g, 0:512], po[:, 0:512])
                        nc.vector.tensor_copy(oq[:, g, 512:1024],
                                              po[:, 512:1024])
                    else:
                        emit_out_group(s, g, oq[:, g, :],
                                       nc.scalar if g % 2 == 0 else nc.vector)
                    # attach kv units round-robin (slabs 5/6 carry 6 units)
                    take = (nu - ui + (GPS - 1 - g)) // (GPS - g)
                    for _ in range(take):
                        emit_kv(units[ui], kq[:, ui, :], vq[:, ui, :])
                        ui += 1
                    if last_slab:
                        # per-group stores for a fast drain
                        eng = _seng() if g < GPS - 1 else nc.sync
                        store_quad(out_d, oq[:, g: g + 1, :], (s * GPS + g) * P,
                                   1, eng)
                t0 = s * SLAB
                if not last_slab:
                    store_quad(out_d, oq, t0, GPS)
                if nu:
                    store_kv_runs(units, kq, vq)

    nc.compile()
    return nc


def _get_nc(**kw):
    key = tuple(sorted(kw.items()))
    if key not in _NC_CACHE:
        _NC_CACHE[key] = build_nc(**kw)
    return _NC_CACHE[key]


def make_in_maps(S, w_spoke, w_hub_priv, w_hub_shared, tag, W_read, cache,
                 latent, Wk, Wv):
    # host-side layout prep only (shard over batch, feature-major transpose,
    # zero-pad to 9*128 rows, slab-major tiling, bf16 downcast)
    bf = ml_dtypes.bfloat16
    NR = NJ * P  # 1152 padded rows

    def slabify(xT_pad):  # [NR, T] -> [NSLAB, P, NJ, SLAB]
        return np.ascontiguousarray(
            xT_pad.reshape(NJ, P, NSLAB, SLAB).transpose(2, 1, 0, 3)
        ).astype(bf)

    wcat = np.concatenate(
        [np.asarray(w_spoke, np.float32), np.asarray(w_hub_priv, np.float32),
         np.asarray(w_hub_shared, np.float32), np.asarray(tag, np.float32)],
        axis=-1)
    S = np.asarray(S, np.float32)

    sT = np.zeros((B, NR, T), np.float32)
    sT[:, :D_STATE, :] = S.transpose(0, 2, 1)
    wT = np.zeros((B, NR, T), np.float32)
    wT[:, :D_STATE, :] = wcat.transpose(0, 2, 1)

    wr_pad = np.zeros((NR, D_MODEL), np.float32)
    wr_pad[:D_STATE] = np.asarray(W_read, np.float32).T
    wrt = np.ascontiguousarray(
        wr_pad.reshape(NJ, P, D_MODEL).transpose(1, 0, 2)).astype(bf)

    latT = np.ascontiguousarray(
        np.asarray(latent, np.float32).transpose(0, 2, 1)).astype(bf)
    wkvt = np.ascontiguousarray(
        np.stack([np.asarray(Wk, np.float32).T, np.asarray(Wv, np.float32).T],
                 axis=1)).astype(bf)
    return [
        {"st": slabify(sT[i]), "wct": slabify(wT[i]), "latt": latT[i],
         "wrt": wrt, "wkvt": wkvt}
        for i in range(N_CORES)
    ]


def kernel(S, w_spoke, w_hub_priv, w_hub_shared, tag, W_read, cache, latent,
           Wk, Wv, **build_kw):
    in_maps = make_in_maps(S, w_spoke, w_hub_priv, w_hub_shared, tag, W_read,
                           cache, latent, Wk, Wv)
    nc = _get_nc(**build_kw)
    res = run_bass_kernel_spmd(nc, in_maps, list(range(N_CORES)))
    out = np.stack([res.results[i]["out"].astype(np.float32)
                    for i in range(N_CORES)])
    k = np.stack([res.results[i]["k"].astype(np.float32)
                  for i in range(N_CORES)])
    v = np.stack([res.results[i]["v"].astype(np.float32)
                  for i in range(N_CORES)])
    return (out, k, v)
